# revision 55
# baseline (speedup 1.0000x reference)
"""Trainium2 Bass kernel for DeepMHCII-style EL_Split_AttMIL model.

Contract: kernel(**inputs) takes FULL unsharded inputs (as produced by
setup_inputs()), returns the FULL (32, 2) float32 output.

Strategy
--------
Data-parallel over bags: 8 cores x 128 instances (= 4 whole bags of 32).
All params replicated. No collectives.

Math reduction (exact, same derivation as the fp32 baseline):
  The interaction convs factor through a per-instance Gram matrix
      G[n, m, p] = sum_e mhc_e[n, m, e] * pep_e[n, p, e]   (34 x 27)
  and each of the three streams is a shared-weight matmul over
  position-shifted copies of G:
      out[n, c, i] = sum_{t, m} W238[(t, m), c] * G[n, m, i + t]
  with the 7 t-blocks ordered as T_ORDER so each conv's support lies in
  one dense 238-row weight matrix (zero-padded outside its k-support).
  BatchNorm folded into weights/biases on the host.
      stream F  : w_cf;  stream R0 : w_cr;  stream R1 : w_cr[:, ::-1]

Device pipeline (fp8 e4m3 + DoubleRow matmuls, 0.5 cycles/row):
  1. HOST builds G and the shifted im2col X directly, appends a ones row
     (bias) + zero row, ships fp8 in DoubleRow pair layout
     [120, 2, NPC*21] (row r = b*120 + p).
  2. Conv = 15 DoubleRow matmuls per 504-col chunk (3 streams x 5
     128-channel blocks); per-block bias rides in the matmul as the
     weight row against the ones row, so evacuation is pure ReLU.
  3. Adjacent out-blocks share one two-bank PSUM tile (ring of 4) -> a
     single ReLU evacuation [128, 2, 504] drains both (fp32 PSUM -> fp8
     SBUF).  Only ScalarE/VectorE have a PSUM port on TRN2, so a credit
     scheduler balances evacuations across those two; GpSimd handles the
     SBUF-side end-stage.  Dedicated xcat/y1 tiles per (chunk, stream)
     avoid WAR fences in the strict-FIFO engine queues; the constant
     ones blocks arrive by DMA.
  4. MLP 640->512 (3 DoubleRow k-pairs; pair 2 = (zeros,kt4)+(bias row)
     against xcat blocks 4 and a constant ones block 5) -> 512->256
     (2 k-pairs, no bias).  Max-pool over the 21 positions is fused into
     the MLP2 PSUM read (one reduce_max per stream over both out-blocks
     on DVE); MLP2 bias+ReLU deferred to the end (valid since
     relu(max(x)+b) = max(relu(x+b))).
  5. Attention MIL tail in fp32 (per-bag softmax over 32 instances),
     sigmoid via exp + reciprocal; output (2, 4) per core.
"""

import sys
from contextlib import ExitStack

import numpy as np
import ml_dtypes

if "/opt/trn_rl_repo" not in sys.path:
    sys.path.insert(0, "/opt/trn_rl_repo")

import concourse.bass as bass
import concourse.bacc as bacc
import concourse.tile as tile
from concourse import mybir
from concourse.bass_utils import run_bass_kernel_spmd

F32 = mybir.dt.float32
BF16 = mybir.dt.bfloat16
F8 = mybir.dt.float8e4
NP_F8 = ml_dtypes.float8_e4m3
NP_BF16 = ml_dtypes.bfloat16
AX = mybir.AxisListType
AF = mybir.ActivationFunctionType
ALU = mybir.AluOpType
DR = mybir.MatmulPerfMode.DoubleRow

# Model constants (hardcoded; must match reference.py)
N, B = 1024, 32
PEP_PAD, L, M, E, VOCAB = 3, 27, 34, 16, 26
CN, KS, OFFS = (128, 256, 256), (3, 5, 7), (2, 1, 0)
LIN = (512, 256)
BN_EPS = 1e-5

NCORES = 8
NPC = N // NCORES          # 128 instances per core
LOUT = 21                  # conv output positions
COLS = NPC * LOUT          # 2688 free columns per core
CCAT = sum(CN)             # 640
H1, H2 = LIN               # 512, 256
BAGS_PER_CORE = 4
BAG = 32

KROWS = 7 * M              # 238 im2col data rows (+ ones row + zero row)
KP = 120                   # partitions in DoubleRow pair layout (240 rows)
FMAX = 504                 # free-dim chunk (24 instances x 21), <= 512 PSUM
PSTRIDE = 512              # psum half-tile stride (bank-aligned)
CHUNKS = [(0, 504), (504, 504), (1008, 504), (1512, 504), (2016, 504),
          (2520, 168)]

# The three convs' t-sets nest; T_ORDER makes every conv's contraction a
# contiguous prefix of the 238-row sequence (j0: 102, j1: 170, j2: 238).
T_ORDER = [2, 3, 4, 1, 5, 0, 6]
T_ROW0 = {t: i * M for i, t in enumerate(T_ORDER)}

# fp8 weight blob column layout: 47 pieces of 256 cols ([p, ktile(2), c]).
OFF_CONV = 0                       # 15 pieces: (st, ob)
OFF_M1F = 15 * 256                 # 12 pieces: (o, i)
OFF_M1R = OFF_M1F + 12 * 256
OFF_M2F = OFF_M1R + 12 * 256       # 4 pieces: (o, i)
OFF_M2R = OFF_M2F + 4 * 256
CW8 = OFF_M2R + 4 * 256            # 12032

# bf16 tail-weight blob + fp32 bias blob
_MOFF = {"ATT1": 0, "ATT2": 512, "WOUT": 516}
CMB = 520
CMISC = 32
BC_L2F, BC_L2R = 0, 2              # MLP2 biases (applied at the end)
BC_ATT1, BC_ATT2, BC_NBOUT = 4, 6, 7


def _pe_table(length, d):
    pos = np.arange(length, dtype=np.float32)[:, None]
    div = np.exp(np.arange(0, d, 2, dtype=np.float32) * (-np.log(10000.0) / d))
    pe = np.zeros((length, d), np.float32)
    pe[:, 0::2] = np.sin(pos * div)
    pe[:, 1::2] = np.cos(pos * div)
    return pe


PE_MHC = _pe_table(M, E)
PE_PEP = _pe_table(100, E)[: L - 2 * PEP_PAD]


def _q8(x):
    return np.clip(x, -240.0, 240.0).astype(NP_F8)


def _conv_w238(p, st):
    """Dense (238, 640) BN-folded conv weight matrix + bias for stream st."""
    tag = "cf" if st == 0 else "cr"
    ch0 = [0, CN[0], CN[0] + CN[1]]
    W238 = np.zeros((KROWS, CCAT), np.float32)
    bias = np.zeros(CCAT, np.float32)
    for j, (C, K, off) in enumerate(zip(CN, KS, OFFS)):
        W = np.asarray(p[f"w_{tag}{j}"], np.float32)
        s = np.asarray(p[f"g_{tag}{j}"], np.float32) / np.sqrt(1.0 + BN_EPS)
        Wp = W * s[:, None, None]
        if st == 2:
            Wp = Wp[:, ::-1]
        bias[ch0[j]:ch0[j] + C] = (np.asarray(p[f"b_{tag}{j}"], np.float32) * s
                                   + np.asarray(p[f"be_{tag}{j}"], np.float32))
        for k in range(K):
            t = off + k
            W238[T_ROW0[t]:T_ROW0[t] + M, ch0[j]:ch0[j] + C] = Wp[:, k, :].T
    return W238, bias


def _mlp_w(p, br, li):
    W = np.asarray(p[f"w_{br}{li}"], np.float32)
    s = np.asarray(p[f"g_{br}{li}"], np.float32) / np.sqrt(1.0 + BN_EPS)
    Wp = W * s[:, None]
    bp = (np.asarray(p[f"b_{br}{li}"], np.float32) * s
          + np.asarray(p[f"be_{br}{li}"], np.float32))
    return Wp, bp


def _pack_lin(Wl, nchunk):
    # Wl (O, I) -> lhsT (I, O) -> (nchunk, 128, O) -> (128, nchunk*O)
    O_, I_ = Wl.shape
    assert I_ == nchunk * 128
    return np.ascontiguousarray(
        Wl.T.reshape(nchunk, 128, O_).transpose(1, 0, 2).reshape(128, nchunk * O_))


def _build_blobs(p):
    """fp8 matmul blob (128, CW8), bf16 tail blob, fp32 bias blob."""
    w8 = np.zeros((128, CW8), np.float32)
    wmb = np.zeros((128, CMB), np.float32)
    misc = np.zeros((128, CMISC), np.float32)

    for st in range(3):
        W238, bias = _conv_w238(p, st)
        # rows 238 = bias (pairs with the ones row of the im2col), 239 = 0
        W240 = np.concatenate(
            [W238, bias[None, :], np.zeros((1, CCAT), np.float32)], axis=0)
        for ob in range(5):
            off = OFF_CONV + (st * 5 + ob) * 256
            blk = W240[:, ob * 128:(ob + 1) * 128]       # (240, 128)
            w8[0:KP, off:off + 128] = blk[0:KP]
            w8[0:KP, off + 128:off + 256] = blk[KP:2 * KP]

    for br, (o1, o2, bc2) in (("lf", (OFF_M1F, OFF_M2F, BC_L2F)),
                              ("lr", (OFF_M1R, OFF_M2R, BC_L2R))):
        W1, b1 = _mlp_w(p, br, 0)
        W2, b2 = _mlp_w(p, br, 1)
        W1T, W2T = W1.T, W2.T                            # (640,512), (512,256)
        for o in range(4):
            for i in range(3):
                off = o1 + (o * 3 + i) * 256
                for rr in range(2):
                    kt = 2 * i + rr
                    if i == 2 and rr == 0:
                        kt = 4                           # pairs with block 4
                    elif i == 2 and rr == 1:
                        # bias row 0 against the constant ones block 5
                        w8[0, off + 128:off + 256] = b1[o * 128:(o + 1) * 128]
                        continue
                    w8[:, off + rr * 128:off + (rr + 1) * 128] = \
                        W1T[kt * 128:(kt + 1) * 128, o * 128:(o + 1) * 128]
        for o in range(2):
            for i in range(2):
                off = o2 + (o * 2 + i) * 256
                for rr in range(2):
                    kt = 2 * i + rr
                    w8[:, off + rr * 128:off + (rr + 1) * 128] = \
                        W2T[kt * 128:(kt + 1) * 128, o * 128:(o + 1) * 128]
        for o in range(2):
            misc[:, bc2 + o] = b2[o * 128:(o + 1) * 128]

    # attention head (bf16 weights, fp32 biases)
    w1 = np.asarray(p["w_att1"], np.float32) / 3.0       # fold mean over 3
    wmb[:, _MOFF["ATT1"]:_MOFF["ATT1"] + 512] = _pack_lin(w1, 2)
    w2 = np.concatenate([np.asarray(p["w_att2"], np.float32)] * 2, axis=0)
    wmb[:, _MOFF["ATT2"]:_MOFF["ATT2"] + 4] = _pack_lin(w2, 2)
    wmb[:, _MOFF["WOUT"]:_MOFF["WOUT"] + 4] = _pack_lin(
        np.asarray(p["w_out"], np.float32), 2)
    misc[:, BC_ATT1] = np.asarray(p["b_att1"], np.float32)[:128]
    misc[:, BC_ATT1 + 1] = np.asarray(p["b_att1"], np.float32)[128:]
    misc[0:2, BC_ATT2] = float(np.asarray(p["b_att2"]).reshape(-1)[0])
    misc[0:2, BC_NBOUT] = -np.asarray(p["b_out"], np.float32).reshape(2)
    return _q8(w8), wmb.astype(NP_BF16), misc


def build_bass():
    nc = bacc.Bacc()
    im_d = nc.declare_dram_parameter("im", [KP, 2 * 6 * PSTRIDE], F8,
                                     isOutput=False)
    w8_d = nc.declare_dram_parameter("w8", [128, CW8], F8, isOutput=False)
    wmb_d = nc.declare_dram_parameter("wmb", [128, CMB], BF16, isOutput=False)
    wm_d = nc.declare_dram_parameter("wm", [128, CMISC], F32, isOutput=False)
    on_d = nc.declare_dram_parameter("ones8", [128, 3 * FMAX], F8,
                                     isOutput=False)
    out_d = nc.declare_dram_parameter("out", [2, BAGS_PER_CORE], F32, isOutput=True)

    with tile.TileContext(nc) as tc:
        with ExitStack() as ctx:
            _emit(ctx, tc, nc, im_d, w8_d, wmb_d, wm_d, on_d, out_d)
    nc.compile()
    return nc


def _emit(ctx, tc, nc, im_d, w8_d, wmb_d, wm_d, on_d, out_d):
    const = ctx.enter_context(tc.tile_pool(name="const", bufs=1))
    psd = ctx.enter_context(tc.tile_pool(name="psd", bufs=4, space="PSUM"))

    im_sb = const.tile([KP, 2 * 6 * PSTRIDE], F8)
    w8_sb = const.tile([128, CW8], F8)
    wmb_sb = const.tile([128, CMB], BF16)
    msb = const.tile([128, CMISC], F32)
    im3 = im_sb.rearrange("p (two c) -> p two c", two=2)
    im3_d = im_d.rearrange("p (two c) -> p two c", two=2)

    # PE warm-up: junk DoubleRow matmuls on a zeroed scratch tile keep the
    # PE continuously busy through the DMA window so the p-state ramp
    # completes before real work arrives.
    scratch = const.tile([128, 512], F8)
    nc.gpsimd.memset(scratch[:], 0.0)
    scr3 = scratch.rearrange("p (two c) -> p two c", two=2)
    for _ in range(14):
        warm_ps = psd.tile([128, 2 * PSTRIDE], F32, tag="ps")
        nc.tensor.matmul(warm_ps[:, 0:252], scr3[:, :, 0:128],
                         scr3[:, :, 0:252],
                         start=True, stop=True, perf_mode=DR)

    # DMA schedule: pieces ordered by first use; the DMA engines resource
    # serializes transfers, so order == arrival order.
    dma_plan = ["im0", "w8c", "on0", "im1", "w8m", "wm", "wmb",
                "im2", "on1", "on2", "on3", "im3", "on4", "on5"]

    def wpiece(off, kp=128):
        return w8_sb[0:kp, off:off + 256].rearrange("p (two c) -> p two c", two=2)

    def bias_col(c):
        return msb[:, c:c + 1]

    def mslice(name, cols):
        o = _MOFF[name]
        return wmb_sb[:, o + cols.start:o + cols.stop]

    # working tiles: one tile per (chunk, stream) — no buffer reuse, so the
    # tile framework never needs WAR fences that would block the strict-FIFO
    # evac engine queues.  The constant ones block (5) of each xcat arrives
    # by DMA (vector engines are the bottleneck; the DMA rings are idle).
    NCH = len(CHUNKS)
    xcat_bufs = [const.tile([128, 18 * FMAX], F8, name=f"xcat_{j}")
                 for j in range(NCH)]
    y1_bufs = [[const.tile([128, 4 * FMAX], F8, name=f"y1_{st}_{j}")
                for j in range(NCH)] for st in range(3)]

    def dma_piece(which):
        if which == "im0":
            nc.sync.dma_start(im3[:, :, 0:512], im3_d[:, :, 0:512])
        elif which == "w8c":
            nc.sync.dma_start(w8_sb[:, OFF_CONV:OFF_M1F],
                              w8_d[:, OFF_CONV:OFF_M1F])
        elif which == "im1":
            nc.sync.dma_start(im3[:, :, 512:1024], im3_d[:, :, 512:1024])
        elif which == "w8m":
            nc.sync.dma_start(w8_sb[:, OFF_M1F:CW8], w8_d[:, OFF_M1F:CW8])
        elif which == "wm":
            nc.sync.dma_start(msb[:], wm_d[:])
        elif which == "wmb":
            nc.sync.dma_start(wmb_sb[:], wmb_d[:])
        elif which == "im2":
            nc.sync.dma_start(im3[:, :, 1024:2048], im3_d[:, :, 1024:2048])
        elif which == "im3":
            nc.sync.dma_start(im3[:, :, 2048:6 * PSTRIDE],
                              im3_d[:, :, 2048:6 * PSTRIDE])
        elif which.startswith("on"):
            ci = int(which[2:])
            nc.sync.dma_start(
                xcat_bufs[ci].rearrange("p (s b f) -> p s b f", s=3,
                                        f=FMAX)[:, :, 5, :],
                on_d[:])

    for piece in dma_plan:
        dma_piece(piece)
    pools = [const.tile([128, 2 * NPC], BF16, name=f"pool{st}") for st in range(3)]
    pools3 = [t.rearrange("p (o n) -> p o n", o=2) for t in pools]
    feat = const.tile([128, 2 * NPC], BF16)
    tmp = const.tile([128, 2 * NPC], BF16)

    # pure-ReLU evacuation.  Only ScalarE (ACT) and VectorE (DVE) have a
    # PSUM port on TRN2, so evacuations balance across those two; GpSimd
    # handles SBUF-side work elsewhere.  Credit costs from the cost model:
    # ACT 1097/pair, 662/single; DVE 1315/pair, 720/single.
    eload = {"A": 0.0, "D": 0.0}
    ecost = {("A", 2): 1097.0, ("A", 1): 662.0,
             ("D", 2): 1370.0, ("D", 1): 750.0}

    def eng_charge(e, ns):
        eload[e] += ns

    def evac(dst, src, width=1):
        e = min(eload, key=lambda k: eload[k] + ecost[(k, width)])
        eng_charge(e, ecost[(e, width)])
        if e == "A":
            nc.scalar.activation(dst, src, AF.Relu)
        else:
            nc.vector.tensor_scalar_max(dst, src, 0.0)

    xcats = {}
    y1s = {}

    def emit_conv_stream(ci, st):
        c0 = ci * PSTRIDE
        F = CHUNKS[ci][1]
        x18 = xcat_bufs[ci].rearrange("p (b f) -> p b f", f=FMAX)

        def dbl(obp):
            def run():
                pj = psd.tile([128, 2 * PSTRIDE], F32, tag="ps", name="pj")
                pj2 = pj.rearrange("p (o f) -> p o f", o=2)
                for h in range(2):
                    ob = 2 * obp + h
                    nc.tensor.matmul(pj2[:, h, 0:F],
                                     wpiece(OFF_CONV + (st * 5 + ob) * 256, KP),
                                     im3[:, :, c0:c0 + F],
                                     start=True, stop=True, perf_mode=DR)
                evac(x18[:, st * 6 + 2 * obp:st * 6 + 2 * obp + 2, 0:F],
                     pj2[:, :, 0:F], width=2)
            return run

        def b4cross():
            # b4 of streams 0 and 1 share one pair tile; dst blocks are
            # 6*FMAX apart in the shared per-chunk xcat tensor
            pj = psd.tile([128, 2 * PSTRIDE], F32, tag="ps", name="pj")
            pj2 = pj.rearrange("p (o f) -> p o f", o=2)
            for h in range(2):
                nc.tensor.matmul(pj2[:, h, 0:F],
                                 wpiece(OFF_CONV + (h * 5 + 4) * 256, KP),
                                 im3[:, :, c0:c0 + F],
                                 start=True, stop=True, perf_mode=DR)
            dst = xcat_bufs[ci].rearrange("p (s x) -> p s x", s=3)
            evac(dst[:, 0:2, 4 * FMAX:4 * FMAX + F], pj2[:, :, 0:F], width=2)

        def single():
            pj = psd.tile([128, 2 * PSTRIDE], F32, tag="ps", name="pj")
            nc.tensor.matmul(pj[:, 0:F],
                             wpiece(OFF_CONV + (2 * 5 + 4) * 256, KP),
                             im3[:, :, c0:c0 + F],
                             start=True, stop=True, perf_mode=DR)
            evac(x18[:, 16, 0:F], pj[:, 0:F], width=1)

        if st == 0:
            return [dbl(0), dbl(1)]
        if st == 1:
            return [dbl(0), dbl(1), b4cross]
        return [dbl(0), dbl(1), single]

    def emit_mlp1_stream(ci, st):
        c0, F = CHUNKS[ci]
        o1 = OFF_M1F if st == 0 else OFF_M1R
        x18 = xcat_bufs[ci].rearrange("p (b f) -> p b f", f=FMAX)
        y13 = y1_bufs[st][ci].rearrange("p (b f) -> p b f", f=FMAX)

        def pair(op_):
            def run():
                p1 = psd.tile([128, 2 * PSTRIDE], F32, tag="ps", name="p1")
                p12 = p1.rearrange("p (o f) -> p o f", o=2)
                for h in range(2):
                    o = 2 * op_ + h
                    for i in range(3):
                        rb = 2 * i if i < 2 else 4   # pair 2: (4, ones)
                        nc.tensor.matmul(p12[:, h, 0:F],
                                         wpiece(o1 + (o * 3 + i) * 256),
                                         x18[:, st * 6 + rb:st * 6 + rb + 2, 0:F],
                                         start=(i == 0), stop=(i == 2),
                                         perf_mode=DR)
                evac(y13[:, 2 * op_:2 * op_ + 2, 0:F], p12[:, :, 0:F], width=2)
            return run

        return [pair(0), pair(1)]

    def emit_mlp2_stream(ci, st):
        c0, F = CHUNKS[ci]
        n0, ninst = c0 // LOUT, F // LOUT
        o2 = OFF_M2F if st == 0 else OFF_M2R
        y13 = y1_bufs[st][ci].rearrange("p (b f) -> p b f", f=FMAX)

        def run():
            p2 = psd.tile([128, 2 * PSTRIDE], F32, tag="ps", name="p2")
            p22 = p2.rearrange("p (o f) -> p o f", o=2)
            for o in range(2):
                for i in range(2):
                    nc.tensor.matmul(p22[:, o, 0:F],
                                     wpiece(o2 + (o * 2 + i) * 256),
                                     y13[:, 2 * i:2 * i + 2, 0:F],
                                     start=(i == 0), stop=(i == 1),
                                     perf_mode=DR)
            # fused max-pool over the 21 positions straight from PSUM (DVE
            # is the only engine with X-axis reduce + a PSUM port)
            eng_charge("D", 1315.0)
            nc.vector.reduce_max(
                pools3[st][:, :, n0:n0 + ninst],
                p22[:, :, 0:F].rearrange("p o (n q) -> p o n q", q=LOUT), AX.X)

        return [run]

    # deferred bias+ReLU + feat accumulation on the pooled features:
    # relu(max(x)+b).  Split by instance columns so the bulk (chunks 0-4)
    # runs on the idle GpSimd engine while chunk 5 is still in flight,
    # leaving only an 8-column sliver for the serial tail.
    t3 = tmp.rearrange("p (o n) -> p o n", o=2)
    f3 = feat.rearrange("p (o n) -> p o n", o=2)

    def end_stage(n0, n1, engs):
        for st in range(3):
            bc2 = BC_L2F if st == 0 else BC_L2R
            for o in range(2):
                eng = engs[(st + o) % len(engs)]
                eng.tensor_scalar(pools3[st][:, o, n0:n1],
                                  pools3[st][:, o, n0:n1],
                                  bias_col(bc2 + o), 0.0, ALU.add, ALU.max)
        engs[0].tensor_add(t3[:, :, n0:n1], pools3[0][:, :, n0:n1],
                           pools3[1][:, :, n0:n1])
        engs[-1].tensor_add(f3[:, :, n0:n1], t3[:, :, n0:n1],
                            pools3[2][:, :, n0:n1])

    # software pipeline: conv runs 2 chunks ahead of the MLPs.  Work is
    # emitted as fine-grained PSUM-group generators, interleaved so each
    # ring slot's reuse distance is maximal.
    def conv_groups(ci):
        for st in range(3):
            for g in emit_conv_stream(ci, st):
                yield g

    def mlp_groups(ci):
        for st in range(3):
            for g in emit_mlp1_stream(ci, st):
                yield g
        for st in range(3):
            for g in emit_mlp2_stream(ci, st):
                yield g

    def interleave(a, b):
        a, b = list(a), list(b)
        # spread the shorter list evenly through the longer
        if len(a) < len(b):
            a, b = b, a
        out = []
        ratio = len(a) / (len(b) + 1e-9) if b else 1e9
        ai = bi = 0
        while ai < len(a) or bi < len(b):
            if bi < len(b) and ai >= ratio * (bi + 1) or ai >= len(a):
                out.append(b[bi]); bi += 1
            else:
                out.append(a[ai]); ai += 1
        return out

    for g in interleave(conv_groups(0), []):
        g()
    for ci in range(len(CHUNKS)):
        nxt = conv_groups(ci + 1) if ci + 1 < len(CHUNKS) else []
        for g in interleave(mlp_groups(ci), nxt):
            g()
        if ci == len(CHUNKS) - 2:
            end_stage(0, 120, [nc.gpsimd])
    end_stage(120, NPC, [nc.vector, nc.gpsimd])



    # ---- attention MIL tail (bf16 matmuls, fp32 softmax math) ----
    att = ctx.enter_context(tc.tile_pool(name="att", bufs=1))

    def mmacc(psum, passes):
        for i, (lh, rh) in enumerate(passes):
            nc.tensor.matmul(psum, lh, rh,
                             start=(i == 0), stop=(i == len(passes) - 1))

    s_w = []
    for w, pl in enumerate(pools):
        psc = psd.tile([2, NPC], F32, tag="ps")
        mmacc(psc, [(mslice("WOUT", slice(kc * 2, kc * 2 + 2)),
                     pl[:, kc * NPC:(kc + 1) * NPC]) for kc in range(2)])
        ew = att.tile([2, NPC], F32, tag=f"ew{w}")
        # sigmoid(z + b) = 1 / (1 + exp(-z - b))
        nc.scalar.activation(ew[:], psc[:], AF.Exp, scale=-1.0,
                             bias=msb[0:2, BC_NBOUT:BC_NBOUT + 1])
        e1 = att.tile([2, NPC], F32, tag=f"e1{w}")
        if w < 2:
            # 1/(2 + 2*ew) = sigmoid/2: folds the later mean-of-two x0.5
            nc.vector.tensor_scalar(e1[:], ew[:], 1.0, 2.0, ALU.add, ALU.mult)
        else:
            nc.vector.tensor_scalar_add(e1[:], ew[:], 1.0)
        sw = att.tile([2, NPC], F32, tag=f"sw{w}")
        nc.vector.reciprocal(sw[:], e1[:])
        s_w.append(sw)

    h_sb = att.tile([128, 2 * NPC], BF16)
    for o in range(2):
        ph = psd.tile([128, PSTRIDE], F32, tag="ps")
        mmacc(ph[:, 0:NPC], [(mslice("ATT1", slice(kc * 256 + o * 128, kc * 256 + (o + 1) * 128)),
                              feat[:, kc * NPC:(kc + 1) * NPC]) for kc in range(2)])
        nc.scalar.activation(h_sb[:, o * NPC:(o + 1) * NPC], ph[:, 0:NPC], AF.Tanh,
                             bias=bias_col(BC_ATT1 + o))

    # a duplicated onto 2 partitions (ATT2 has 2 identical output cols)
    pa = psd.tile([2, NPC], F32, tag="ps")
    mmacc(pa, [(mslice("ATT2", slice(kc * 2, kc * 2 + 2)),
                h_sb[:, kc * NPC:(kc + 1) * NPC]) for kc in range(2)])
    ex2 = att.tile([2, NPC], F32)
    nc.scalar.activation(ex2[:], pa[:], AF.Exp,
                         bias=msb[0:2, BC_ATT2:BC_ATT2 + 1])

    hs = att.tile([2, NPC], F32)
    nc.vector.tensor_add(hs[:], s_w[0][:], s_w[1][:])
    smax = att.tile([2, NPC], F32)
    nc.vector.tensor_tensor(smax[:], hs[:], s_w[2][:], ALU.max)

    p2 = att.tile([2, NPC], F32)
    nc.vector.tensor_mul(p2[:], smax[:], ex2[:])

    pb = att.tile([2, BAGS_PER_CORE], F32)
    nc.vector.tensor_reduce(pb[:], p2[:].rearrange("p (b i) -> p b i", i=BAG),
                            AX.X, ALU.add)
    eb = att.tile([2, BAGS_PER_CORE], F32)
    nc.vector.tensor_reduce(eb[:], ex2[:].rearrange("p (b i) -> p b i", i=BAG),
                            AX.X, ALU.add)
    rb = att.tile([2, BAGS_PER_CORE], F32)
    nc.vector.reciprocal(rb[:], eb[:])
    osb = att.tile([2, BAGS_PER_CORE], F32)
    nc.vector.tensor_mul(osb[:], pb[:], rb[:])
    nc.sync.dma_start(out_d[:], osb[:])


_CACHED = {}


def _get_nc():
    if "nc" not in _CACHED:
        _CACHED["nc"] = build_bass()
    return _CACHED["nc"]


def _host_prep(inputs):
    p = {k: np.asarray(v) for k, v in inputs.items()}
    assert int(p["inverse"]) == 1
    bs = np.asarray(p["bags_size"]).reshape(-1)
    assert bs.shape[0] == B and np.all(bs == N // B), "kernel compiled for equal bags of 32"

    pep_e = p["emb_pep"].astype(np.float32)[p["peptide_x"]]       # (N, 27, 16)
    pep_e[:, PEP_PAD:L - PEP_PAD] += PE_PEP
    mhc_e = p["emb_mhc"].astype(np.float32)[p["mhc_x"]] + PE_MHC  # (N, 34, 16)

    # G and shifted im2col X[r, n, q] = G[n, m_r, q + t_r]; row 238 = ones
    # (bias), row 239 = zeros.
    G = np.einsum("nme,npe->nmp", mhc_e, pep_e).astype(np.float32)
    X = np.zeros((2 * KP, N, LOUT), np.float32)
    for r in range(KROWS):
        t, m = T_ORDER[r // M], r % M
        X[r] = G[:, m, t:t + LOUT]
    X[KROWS] = 1.0
    X8 = _q8(X)

    w8, wmb, misc = _build_blobs(p)
    ones8 = np.ones((128, 3 * FMAX), NP_F8)
    in_maps = []
    for c in range(NCORES):
        xc = X8[:, c * NPC:(c + 1) * NPC, :].reshape(2 * KP, COLS)
        xp = np.zeros((2 * KP, 6, 512), NP_F8)
        for ci, (c0, F) in enumerate(CHUNKS):
            xp[:, ci, 0:F] = xc[:, c0:c0 + F]
        im = np.ascontiguousarray(
            xp.reshape(2, KP, 6 * 512).transpose(1, 0, 2).reshape(KP, -1))
        in_maps.append({"im": im, "w8": w8, "wmb": wmb, "wm": misc,
                        "ones8": ones8})
    return in_maps


def kernel(**inputs) -> np.ndarray:
    in_maps = _host_prep(inputs)
    nc = _get_nc()
    res = run_bass_kernel_spmd(nc, in_maps, core_ids=list(range(NCORES)))
    out = np.empty((B, 2), np.float32)
    for c in range(NCORES):
        out[c * BAGS_PER_CORE:(c + 1) * BAGS_PER_CORE] = res.results[c]["out"].T
    return out
